# revision 41
# baseline (speedup 1.0000x reference)
"""Trainium2 Bass kernel for nn_MultiHeadAttention (b=2, n=4096, d=512, h=8, hd=64).

Sharding: 8 cores; core c handles batch b=c//4 and head pair j=c%4
(heads 2j, 2j+1). Tensor-parallel heads: each core computes a partial
output-projection y_part (f16); host sums the 4 partials per batch and
adds bo.

Per-core pipeline (16-bit datapaths, exp split across two engines):
  x[b] transposed ON HOST -> xT [128, 4dc, 4096] bf16, DMA'd straight to
    SBUF (no PE transpose / PSUM round trip in phase 1)
  Q/K projected bf16 for both heads at once, kept f32r in
    top/bottom-identical replicated layouts so consecutive K=64 scoresT
    matmuls can run on opposite 64-row groups of the PE array; the row
    half is assigned by ISSUE parity, so every adjacent pair of score
    MMs streams concurrently (measured ~2x on HW)
  V copied to fp16, PE-transposed into fp16 ones-augmented v_aug
    (M=65; row 64 of attn@v accumulates the softmax sums)
  scoresT supers of TWO chunks -> 2-bank PSUM tiles x3 bufs (deeper
    score->exp->attnv pipelining than the old 3-chunk x2 layout), fp16
    expT via EITHER
    - ACT: one Exp ACTIVATE (scale 0.125 folded), fp16 out, or
    - DVE: Schraudolph fast-exp in ONE tensor_scalar f32->int16
      (bits = s*0.125*1024/ln2 + 15*1024 - 15), bitcast to fp16
    per-unit engine chosen by EXP_PAT (ACT-heavy: ACT is 1.25x faster
    per element and carries the yb drains; DVE carries scr drains)
  attn@v: fp16 x fp16 matmuls accumulating [65, 512] in PSUM
  per-q-block normalize: softmax sums bounce through DRAM to spread
    512 values over 128 partitions, reciprocal at [128,4], bounce back
    to a partition-broadcast; y_part = OT.T @ WoT deferred one q-block;
    the ot2*recip multiply runs on gpsimd (Pool) which is otherwise idle
    (it has no PSUM port so it can't help with exp or drains)

Scheduling (the HW pipeline is latency-bound, not throughput-bound):
  - the two heads are INTERLEAVED super-by-super in phase 2 (independent
    dependency chains keep the PE busy while the other head's exp is in
    flight)
  - phase 2 is software-pipelined with lookahead LOOKAHEAD
  - finalize(qb-1) is issued AFTER sweep_tail(1,qb)'s drains so its psy
    tiles reuse the two just-freed ps_o PSUM slots instead of stalling
    against live accumulators
"""

import numpy as np

B, N, D, H, HD = 2, 4096, 512, 8, 64
NT = N // 128          # 32 n-tiles
NBLK = N // 512        # 8 n-blocks
KC = N // 128          # 32 key chunks
QB = N // 512          # 8 q-blocks
SUPW = 2               # chunks per super


def _supers(unpair=False):
    if not unpair:
        return [[c, c + 1] for c in range(0, KC, 2)]
    # adjacent score MMs get SAME-parity chunks -> no half-array pairing
    out = []
    for c in range(0, KC, 4):
        out.append([c, c + 2])
        out.append([c + 1, c + 3])
    return out

# exp engine per unit, cycling: 'A' = ACT activation, 'D' = DVE fast-exp
EXP_PAT = "ADADADADADADADADA"   # 9A:8D per 17 units ~ 0.53 ACT share
# EXP_SPLIT: every unit's exp runs on BOTH engines (ACT cols [0,AQ), DVE
# rest) -> halves the scores->exp->attnv round-trip latency that gates the
# PSUM score-buffer ring, at the cost of one extra instruction per unit
EXP_SPLIT = False
AQ = 288           # ACT column share under EXP_SPLIT (DVE is ~1.25x slower)
YB_PAT = "AD"      # output-projection PSUM->SBUF drain engines, cycling
SCR_PAT = "D"      # attn-out PSUM->SBUF drain engines, cycling
LOOKAHEAD = 3
MUL_ON_POOL = True  # ot2 *= recip on gpsimd
# fast-exp constants: bits = rne(s * 0.125/ln2 * 1024 + 15*1024 + CORR)
FEXP_A = float(0.125 / np.log(2.0) * 1024.0)
FEXP_B = float(15.0 * 1024.0 - 15.0)

_CACHE = {}
ABLATE = "base"  # timing-ablation knob, used only by ablate.py


def _build_nc(loop_n=None):
    """Build the SPMD kernel. loop_n wraps the body in a hardware For loop
    (used only for timing amplification, never for the graded path)."""
    import contextlib

    import concourse.bass as bass
    import concourse.mybir as mybir
    import concourse.tile as tile
    from concourse import bacc

    F32 = mybir.dt.float32
    F32R = mybir.dt.float32r
    BF16 = mybir.dt.bfloat16
    F16 = mybir.dt.float16
    I16 = mybir.dt.int16
    EXP = mybir.ActivationFunctionType.Exp
    MULT = mybir.AluOpType.mult
    ADD = mybir.AluOpType.add

    nc = bacc.Bacc("TRN2", target_bir_lowering=False, debug=False, num_devices=8)

    xT_d = nc.dram_tensor("xT", [128, 4, N], BF16, kind="ExternalInput")
    w_d = {}
    for nm in ("wq2", "wk2", "wv"):
        w_d[nm] = nc.dram_tensor(nm, [128, 4, 128], BF16, kind="ExternalInput")
    woT_d = nc.dram_tensor("woT", [128, 512], F16, kind="ExternalInput")
    ident_d = nc.dram_tensor("ident", [128, 128], F16, kind="ExternalInput")
    y_d = nc.dram_tensor("y_part", [N, D], F16, kind="ExternalOutput")
    sums_dram = nc.dram_tensor("sums_scratch", [2, N], F16, kind="Internal")
    recip_dram = nc.dram_tensor("recip_scratch", [2, N], F16, kind="Internal")

    with tile.TileContext(nc) as tc:
        with (
            tc.tile_pool(name="singles", bufs=1) as singles,
            tc.tile_pool(name="sb_vt", bufs=2) as sb_vt,
            tc.tile_pool(name="sb_exp", bufs=6) as sb_exp,
            tc.tile_pool(name="sb_rsp", bufs=2) as sb_rsp,
            tc.tile_pool(name="sb_y", bufs=3) as sb_y,
            tc.tile_pool(name="ps3", bufs=3, space="PSUM") as ps3,
            tc.tile_pool(name="ps1", bufs=2, space="PSUM") as ps1,
        ):
            loop_ctx = (
                tc.For_i(0, loop_n, 1) if loop_n else contextlib.nullcontext()
            )
            with loop_ctx:
                # xT block 0 first: everything in phase 1 waits on it
                xT_all = singles.tile([128, 4, N], BF16, tag="xT", name="xT_all")
                nc.sync.dma_start(
                    out=xT_all[:, :, 0:512], in_=xT_d.ap()[:, :, 0:512]
                )
                ident = singles.tile([128, 128], F16)
                nc.sync.dma_start(out=ident, in_=ident_d.ap())
                # warm the ACT Exp table while phase 1 runs
                warm = singles.tile([1, 1], F32)
                nc.scalar.activation(out=warm, in_=ident[0:1, 0:1], func=EXP)
                wt = {}
                for nm in ("wq2", "wk2", "wv"):
                    wt[nm] = singles.tile(
                        [128, 4, 128], BF16, tag=f"w_{nm}", name=f"wt_{nm}"
                    )
                    nc.sync.dma_start(out=wt[nm], in_=w_d[nm].ap())
                woT = singles.tile([128, 512], F16)
                nc.sync.dma_start(out=woT, in_=woT_d.ap())

                qrep = [singles.tile([128, N], F16, tag=f"qrep{h}", name=f"qrep{h}")
                        for h in range(2)]
                krep = [singles.tile([128, N], F16, tag=f"krep{h}", name=f"krep{h}")
                        for h in range(2)]
                v_aug = [singles.tile([128, KC, 65], F16, tag=f"vaug{h}",
                                      name=f"vaug{h}") for h in range(2)]
                ot2 = singles.tile([128, N], F16)
                recip_b = singles.tile([128, N], F16)

                # ones column of v_aug (row 64 of attn@v = softmax sums)
                for h in range(2):
                    nc.vector.memset(v_aug[h][:, :, HD:65], 1.0)

                # -------- engine-cycling helpers --------
                def eng_copy(pat_state, pat):
                    e = pat[pat_state[0] % len(pat)]
                    pat_state[0] += 1
                    if e == "A":
                        return lambda out, in_: nc.scalar.copy(out=out, in_=in_)
                    return lambda out, in_: nc.vector.tensor_copy(out=out, in_=in_)

                yb_state = [0]
                scr_state = [0]

                # -------- attention helpers --------
                def norm_mul(qb, on_pool=MUL_ON_POOL):
                    qs = slice(qb * 512, (qb + 1) * 512)
                    eng = nc.gpsimd if on_pool else nc.vector
                    eng.tensor_mul(ot2[:, qs], ot2[:, qs], recip_b[:, qs])

                def finalize_nt(nt, from_ps3=False, yb_pat=None):
                    if from_ps3:
                        psy = ps3.tile([128, SUPW, 512], F32, tag="ps_s",
                                       name="psy3")[:, 0, :]
                    else:
                        psy = ps1.tile([128, 512], F32, tag="psA", name="psy")
                    nc.tensor.matmul(
                        psy, ot2[:, nt * 128:(nt + 1) * 128], woT,
                        start=True, stop=True,
                    )
                    yb = sb_y.tile([128, 512], F16, tag="yb", name="yb")
                    eng_copy(yb_state, yb_pat or YB_PAT)(out=yb, in_=psy)
                    nc.sync.dma_start(
                        out=y_d.ap()[nt * 128:(nt + 1) * 128, :], in_=yb
                    )

                def finalize(qb, mul=True, mul_on_pool=MUL_ON_POOL,
                             from_ps3=False):
                    if mul:
                        norm_mul(qb, mul_on_pool)
                    for nt in range(4 * qb, 4 * qb + 4):
                        finalize_nt(nt, from_ps3)

                def scores_exp(h, qb, chunks, use_dve):
                    qs = slice(qb * 512, (qb + 1) * 512)
                    ps_s = ps3.tile([128, SUPW, 512], F32, tag="ps_s", name="ps_s")
                    w = len(chunks)
                    for i, c in enumerate(chunks):
                        # fixed parity c%2: chunks are processed in order, so
                        # adjacent score MMs still land on opposite PE halves
                        # (the pairing trick), but each K chunk now lives in
                        # ONE half of krep -> no K replication needed.
                        p = c % 2
                        half = slice(p * 64, p * 64 + 64)
                        nc.tensor.matmul(
                            ps_s[:, i, :],
                            krep[h][half, c * 128:(c + 1) * 128],
                            qrep[h][half, qs],
                            start=True, stop=True,
                        )
                    expT = sb_exp.tile([128, SUPW, 512], F16, tag="expT",
                                       name="expT")
                    sl = slice(0, 1) if ABLATE == "tiny_exp" else slice(0, 512)
                    if use_dve and ABLATE != "all_act":
                        nc.vector.tensor_scalar(
                            out=expT.bitcast(I16)[:, 0:w, sl],
                            in0=ps_s[:, 0:w, sl],
                            scalar1=FEXP_A, scalar2=FEXP_B, op0=MULT, op1=ADD,
                        )
                    else:
                        nc.scalar.activation(
                            out=expT[:, 0:w, sl], in_=ps_s[:, 0:w, sl],
                            func=EXP, scale=0.125,
                        )
                    return expT

                def attnv(h, qb, ps_o, expT, chunks):
                    for i, c in enumerate(chunks):
                        if ABLATE == "no_attnv" and c > 0:
                            continue
                        nc.tensor.matmul(
                            ps_o[0:65, :], v_aug[h][:, c, :], expT[:, i, :],
                            start=(c == 0),
                            stop=(c == (0 if ABLATE == "no_attnv" else KC - 1)),
                        )

                def sweep_tail(h, qb, ps_o):
                    qs = slice(qb * 512, (qb + 1) * 512)
                    scr = sb_exp.tile([65, 512], F16, tag="scr", name="scr")
                    # scale by 1/16 so the unnormalized numerator fits f16
                    # (dominant-key rows reach ~1e5); the sums row is scaled
                    # identically, so its reciprocal cancels the factor
                    e = SCR_PAT[scr_state[0] % len(SCR_PAT)]
                    scr_state[0] += 1
                    if e == "A":
                        nc.scalar.activation(
                            out=scr, in_=ps_o[0:65, :],
                            func=mybir.ActivationFunctionType.Copy, scale=0.0625,
                        )
                    else:
                        nc.vector.tensor_scalar_mul(
                            out=scr, in0=ps_o[0:65, :], scalar1=0.0625
                        )
                    nc.sync.dma_start(
                        out=ot2[h * 64:(h + 1) * 64, qs], in_=scr[0:64, :]
                    )
                    # softmax sums: bounce via DRAM to spread the 512 values
                    # across 128 partitions (a [1,512] reciprocal would use a
                    # single DVE lane: measured 3.3us each), take the
                    # reciprocal at [128,4], bounce again to broadcast across
                    # partitions.
                    rrow = scr[64:65, :]
                    nc.sync.dma_start(out=sums_dram.ap()[h:h + 1, qs], in_=rrow)
                    rsp = sb_rsp.tile([128, 4], F16, tag="rsp", name="rsp")
                    rs_ap = bass.AP(
                        tensor=sums_dram, offset=h * N + qb * 512,
                        ap=[[4, 128], [1, 4]],
                    )
                    nc.sync.dma_start(out=rsp, in_=rs_ap)
                    with nc.allow_low_precision("f16 softmax-sum recip: 1e-3 "
                                                "rel err is within budget"):
                        nc.vector.reciprocal(out=rsp, in_=rsp)
                    rd_ap = bass.AP(
                        tensor=recip_dram, offset=h * N + qb * 512,
                        ap=[[4, 128], [1, 4]],
                    )
                    nc.sync.dma_start(out=rd_ap, in_=rsp)
                    rb = bass.AP(
                        tensor=recip_dram, offset=h * N + qb * 512,
                        ap=[[0, 64], [1, 512]],
                    )
                    nc.sync.dma_start(out=recip_b[h * 64:(h + 1) * 64, qs], in_=rb)
                    # psy reuses the two ps_o slots both heads just freed.
                    # finalize(0) is HELD BACK until the drain tail: it is PE
                    # work with long-satisfied deps, so it fills the dead time
                    # while the last q-block's softmax-sum recip chain (4
                    # serial DMAs) is in flight. Its normalize-multiply is
                    # issued early (here) so the tail sees pure PE work.
                    if h == 1:
                        if qb == 0:
                            norm_mul(0)
                        elif qb > 1:
                            finalize(qb - 1)

                # ---- phase 2 unit list (issued interleaved with phase 1) ----
                SUPERS = _supers(unpair=(ABLATE == "unpair"))
                units = []
                ui_pat = 0
                n_units = QB * len(SUPERS) * 2
                qb_range = [] if ABLATE == "phase1_only" else range(QB)
                for qb in qb_range:
                    for si, chunks in enumerate(SUPERS):
                        for h in range(2):
                            use_dve = EXP_PAT[ui_pat % len(EXP_PAT)] == "D"
                            # the last few units' exps go to ACT so DVE is
                            # free for the final sweep's scr/recip chain
                            if ui_pat >= n_units - 4:
                                use_dve = False
                            units.append(
                                (h, qb, chunks, si == len(SUPERS) - 1, use_dve)
                            )
                            ui_pat += 1

                ps_o_cur = {}
                pending = []

                def flush_pending():
                    h, qb, chunks, last, expT = pending.pop(0)
                    if (h, qb) not in ps_o_cur:
                        ps_o_cur[(h, qb)] = ps1.tile(
                            [128, 512], F32, tag="psA", name="ps_o"
                        )
                    attnv(h, qb, ps_o_cur[(h, qb)], expT, chunks)
                    if last:
                        sweep_tail(h, qb, ps_o_cur.pop((h, qb)))

                def issue_unit(u):
                    h, qb, chunks, last, use_dve = u
                    expT = scores_exp(h, qb, chunks, use_dve)
                    if len(pending) >= LOOKAHEAD:
                        flush_pending()
                    pending.append((h, qb, chunks, last, expT))

                def issue_pair(u1, u2):
                    # both heads' score MMs back-to-back (4 MMs alternating
                    # row halves, every Ldweights pulls ahead), then two
                    # attnv batches -> half as many score<->attnv row-group
                    # transitions on the PE
                    h1, qb1, chunks1, last1, dve1 = u1
                    h2, qb2, chunks2, last2, dve2 = u2
                    e1 = scores_exp(h1, qb1, chunks1, dve1)
                    e2 = scores_exp(h2, qb2, chunks2, dve2)
                    for _ in range(2):
                        if len(pending) >= LOOKAHEAD:
                            flush_pending()
                    pending.append((h1, qb1, chunks1, last1, e1))
                    pending.append((h2, qb2, chunks2, last2, e2))

                # unit readiness: block holding its q-block and k/v chunks
                def ready_block(u):
                    h, qb, chunks, last, use_dve = u
                    return max(qb, max(chunks) // 4)

                # ---- phase 1: load host-transposed x, project ----
                # Units whose inputs finished a block ago are interleaved
                # between the projection blocks: phase 2 is PE-hungry and
                # phase 1 is DMA/copy-latency-bound, so the early q-block-0
                # sweeps fill phase 1's stalls. (One block of slack so unit
                # score MMs never head-of-line-block the next projection.)
                ui = 0
                for jj in range(NBLK):
                    ns = slice(jj * 512, (jj + 1) * 512)
                    if jj > 0:
                        nc.sync.dma_start(
                            out=xT_all[:, :, ns], in_=xT_d.ap()[:, :, ns]
                        )
                    # Q and K for both heads. Q is replicated into both
                    # krep-halves by TWO ACT copies from the same PSUM rows
                    # (no SBUF->SBUF DMA); K needs no replication at all:
                    # chunk c is only ever read at partition-half c%2, so two
                    # strided DVE copies place even chunks in the top half and
                    # odd chunks in the bottom half.
                    for pi, (nm, rep) in enumerate((("wq2", qrep), ("wk2", krep))):
                        psp = ps3.tile([128, SUPW, 512], F32, tag="ps_s",
                                       name="psp")
                        for dc in range(4):
                            nc.tensor.matmul(
                                psp[:, 0, :], wt[nm][:, dc, :], xT_all[:, dc, ns],
                                start=(dc == 0), stop=(dc == 3),
                            )
                        if nm == "wq2":
                            for h in range(2):
                                src = psp[h * 64:(h + 1) * 64, 0, :]
                                nc.scalar.copy(out=rep[h][0:64, ns], in_=src)
                                nc.scalar.copy(out=rep[h][64:128, ns], in_=src)
                        else:
                            # block jj covers chunks 4jj..4jj+3: even chunks
                            # at cols {0:128, 256:384}, odd at {128:256,
                            # 384:512} within the block
                            for h in range(2):
                                srcs = psp[h * 64:(h + 1) * 64, 0, :].rearrange(
                                    "p (a b c) -> p a b c", a=2, b=2, c=128)
                                dst = rep[h][:, ns].rearrange(
                                    "p (a b c) -> p a b c", a=2, b=2, c=128)
                                nc.vector.tensor_copy(
                                    out=dst[0:64, :, 0, :], in_=srcs[:, :, 0, :]
                                )
                                nc.vector.tensor_copy(
                                    out=dst[64:128, :, 1, :], in_=srcs[:, :, 1, :]
                                )
                    # V for both heads, fp16, then transpose into v_aug
                    psp = ps3.tile([128, SUPW, 512], F32, tag="ps_s", name="psp_v")
                    for dc in range(4):
                        nc.tensor.matmul(
                            psp[:, 0, :], wt["wv"][:, dc, :], xT_all[:, dc, ns],
                            start=(dc == 0), stop=(dc == 3),
                        )
                    vt_blk = sb_vt.tile([128, 512], F16, tag="vt", name="vt_blk")
                    nc.vector.tensor_copy(out=vt_blk, in_=psp[:, 0, :])
                    for h in range(2):
                        psv = ps3.tile([128, 4, 64], F16, tag="ps_s", name="psv")
                        for tt in range(4):
                            nc.tensor.transpose(
                                psv[:, tt, :],
                                vt_blk[h * 64:(h + 1) * 64, tt * 128:(tt + 1) * 128],
                                ident[h * 64:(h + 1) * 64, h * 64:(h + 1) * 64],
                            )
                        nc.vector.tensor_copy(
                            out=v_aug[h][:, 4 * jj:4 * jj + 4, 0:HD], in_=psv
                        )

                    # phase 2 units whose inputs completed a block ago
                    # (units come in (h0,h1) pairs with identical readiness)
                    while (ui + 1 < len(units)
                           and ready_block(units[ui + 1]) <= jj - 1):
                        issue_pair(units[ui], units[ui + 1])
                        ui += 2

                # ---- phase 2: remaining sweeps, software-pipelined on PE ----
                # Heads interleaved (h innermost): independent dependency
                # chains keep the PE busy while the other head's exp is in
                # flight.
                while ui + 1 < len(units):
                    issue_pair(units[ui], units[ui + 1])
                    ui += 2
                while ui < len(units):
                    issue_unit(units[ui])
                    ui += 1
                # drain: interleave finalize(0)'s held-back output-projection
                # matmuls (deps long satisfied; psy tiles borrowed from the
                # now-draining ps3 scores ring) between the final flushes so
                # the PE stays busy while the last exps/sweeps are in flight.
                # fill yb copies stay OFF DVE so the last sweeps' scr/recip
                # chain isn't queued behind them
                fill = [
                    (lambda nt=nt: finalize_nt(nt, from_ps3=True, yb_pat="A"))
                    for nt in range(4)
                ] if ABLATE != "phase1_only" else []
                while pending:
                    flush_pending()
                    if fill:
                        fill.pop(0)()
                while fill:
                    fill.pop(0)()
                if ABLATE != "phase1_only":
                    finalize(QB - 1, mul_on_pool=False)

    nc.compile()
    return nc


def _prep_in_maps(x, Wq, Wk, Wv, Wo):
    import ml_dtypes

    bf16 = ml_dtypes.bfloat16
    x = np.asarray(x, dtype=np.float32)
    Wq = np.asarray(Wq, dtype=np.float32)
    Wk = np.asarray(Wk, dtype=np.float32)
    Wv = np.asarray(Wv, dtype=np.float32)
    Wo = np.asarray(Wo, dtype=np.float32)
    ident = np.eye(128, dtype=np.float16)
    in_maps = []
    for c in range(8):
        b, j = c // 4, c % 4
        rows = slice(128 * j, 128 * (j + 1))
        # xT layout [p, dc, n] with element [p, dc, n] = x[b][n, dc*128+p]
        xT = np.ascontiguousarray(
            x[b].T.reshape(4, 128, N).transpose(1, 0, 2)
        ).astype(bf16)
        m = {
            "xT": xT,
            "ident": ident,
            "woT": np.ascontiguousarray(Wo[:, rows].T).astype(np.float16),
        }
        for nm, W in (("wq2", Wq), ("wk2", Wk), ("wv", Wv)):
            A = W[rows]                     # [128, 512] rows = h1(64) | h2(64)
            # lhsT layout [p=d-within-chunk, c=d-chunk, k=out-col], contiguous
            m[nm] = np.ascontiguousarray(
                A.reshape(128, 4, 128).transpose(2, 1, 0)
            ).astype(bf16)
        in_maps.append(m)
    return in_maps


def _make_runner(nc):
    """Persistent jitted SPMD executor (mirrors bass2jax.run_bass_via_pjrt)
    so repeat kernel() calls skip re-tracing/re-jitting."""
    import jax
    from jax.sharding import Mesh, PartitionSpec
    from jax.experimental.shard_map import shard_map
    import concourse.mybir as mybir
    from concourse import bass2jax

    bass2jax.install_neuronx_cc_hook()
    n_cores = 8
    partition_name = nc.partition_id_tensor.name if nc.partition_id_tensor else None
    in_names, out_names, out_avals, zero_shapes = [], [], [], []
    for alloc in nc.m.functions[0].allocations:
        if not isinstance(alloc, mybir.MemoryLocationSet):
            continue
        name = alloc.memorylocations[0].name
        if alloc.kind == "ExternalInput":
            if name != partition_name:
                in_names.append(name)
        elif alloc.kind == "ExternalOutput":
            shape = tuple(alloc.tensor_shape)
            dtype = mybir.dt.np(alloc.dtype)
            out_names.append(name)
            out_avals.append(jax.core.ShapedArray(shape, dtype))
            zero_shapes.append((shape, dtype))
    n_params = len(in_names)
    all_names = in_names + out_names
    if partition_name is not None:
        all_names = all_names + [partition_name]

    def _body(*args):
        operands = list(args)
        if partition_name is not None:
            operands.append(bass2jax.partition_id_tensor())
        return tuple(bass2jax._bass_exec_p.bind(
            *operands,
            out_avals=tuple(out_avals),
            in_names=tuple(all_names),
            out_names=tuple(out_names),
            lowering_input_output_aliases=(),
            sim_require_finite=True,
            sim_require_nnan=True,
            nc=nc,
        ))

    mesh = Mesh(np.asarray(jax.devices()[:n_cores]), ("core",))
    n_outs = len(out_names)
    sharded = jax.jit(
        shard_map(
            _body, mesh=mesh,
            in_specs=(PartitionSpec("core"),) * (n_params + n_outs),
            out_specs=(PartitionSpec("core"),) * n_outs,
            check_rep=False,
        ),
        keep_unused=True,
    )
    zeros = [np.zeros((n_cores * s[0], *s[1:]), d) for s, d in zero_shapes]

    def run(in_maps):
        concat_in = [
            np.concatenate([np.asarray(in_maps[c][nm]) for c in range(n_cores)],
                           axis=0)
            for nm in in_names
        ]
        outs = sharded(*concat_in, *zeros)
        arr = np.asarray(outs[out_names.index("y_part")])
        return arr.reshape(n_cores, N, D)

    return run


def kernel(x, Wq, Wk, Wv, Wo, bo):
    if "nc" not in _CACHE:
        _CACHE["nc"] = _build_nc()
    nc = _CACHE["nc"]
    in_maps = _prep_in_maps(x, Wq, Wk, Wv, Wo)
    try:
        if "runner" not in _CACHE:
            _CACHE["runner"] = _make_runner(nc)
        parts = _CACHE["runner"](in_maps)
    except Exception:
        from concourse.bass_utils import run_bass_kernel_spmd
        res = run_bass_kernel_spmd(nc, in_maps, core_ids=list(range(8)))
        parts = np.stack([res.results[c]["y_part"] for c in range(8)])
    y = np.zeros((B, N, D), np.float32)
    for c in range(8):
        y[c // 4] += np.asarray(parts[c], dtype=np.float32)
    y += np.asarray(bo, dtype=np.float32)[None, None, :]
    return y


# revision 42
# speedup vs baseline: 1.0695x; 1.0695x over previous
"""Trainium2 Bass kernel for nn_MultiHeadAttention (b=2, n=4096, d=512, h=8, hd=64).

Sharding: 8 cores; core c handles batch b=c//4 and head pair j=c%4
(heads 2j, 2j+1). Tensor-parallel heads: each core computes a partial
output-projection y_part (f16); host sums the 4 partials per batch and
adds bo.

HW model (established by ablation timing, not the cost-model sim):
  - phase 2 is PE-BOUND: ~1100 512-col matmuls at ~213ns stream +
    ~40-70ns issue/LS overhead each. exp is nearly free (tiny_exp
    ablation: -27us) and all_act ~= split, so ACT/DVE loads don't bind.
  - PE MATMULs never execute concurrently ("pairing" is a myth — the PE
    reorder window only pulls LDWEIGHTS ahead, MATMULs stay in program
    order). But the LDWEIGHTS pull-ahead needs non-conflicting row
    groups: the unpair ablation (same-half consecutive score MMs) costs
    +111us. Hence the strict row-half ALTERNATION below.
  - scores use K=64 (half the array rows); alternating chunks between
    partition halves (chunk c lives at half c%2) keeps every score
    LDWEIGHTS pull-ahead eligible AND removes K replication entirely.

Per-core pipeline (16-bit datapaths):
  x[b] transposed ON HOST -> xT [128, 4dc, 4096] bf16, DMA'd straight to
    SBUF (no PE transpose / PSUM round trip in phase 1)
  Q/K projected bf16, drained to f16 qrep (replicated to both halves by
    two ACT copies) / f16 krep (parity-placed by strided DVE copies);
    f16 weights enable FWL and avoid the f32r LDWEIGHTS hazards
  V copied to fp16, PE-transposed into fp16 ones-augmented v_aug
    (M=65; row 64 of attn@v accumulates the softmax sums)
  scoresT supers of TWO chunks -> 2-bank PSUM tiles x3 bufs; fp16 expT
    via EITHER
    - ACT: one Exp ACTIVATE (scale 0.125 folded), fp16 out, or
    - DVE: Schraudolph fast-exp in ONE tensor_scalar f32->int16
      (bits = s*0.125*1024/ln2 + 15*1024 - 15), bitcast to fp16
    per-unit engine chosen by EXP_PAT
  attn@v: fp16 x fp16 matmuls accumulating [65, 512] f32 in PSUM
  per-q-block normalize: ps_o drained x1/16 to f16 (unnormalized
    numerator can reach ~1e5 > f16 max; sums row scales identically so
    the reciprocal cancels the 1/16), softmax sums bounce through DRAM
    (f16) to spread 512 values over 128 partitions, reciprocal at
    [128,4], bounce back to a partition-broadcast; y = OT.T @ WoT (f16)
    deferred one q-block; the ot2*recip multiply runs on gpsimd (Pool)
    which is otherwise idle (no PSUM port, so it can't help elsewhere)

Scheduling:
  - phase 2 units are interleaved INTO phase 1: each projection block
    enables 4 q-block-0 units (issued one block later so they never
    head-of-line-block the next projection); phase 1 alone is ~62us on
    HW and mostly DMA/copy latency, which phase-2 PE work now fills
  - units issued in (h0,h1) PAIRS: 4 score MMs back-to-back (alternating
    row halves), then two attnv batches -> fewer score<->attnv row-group
    transitions where LDWEIGHTS cannot pull ahead
  - finalize(qb-1) issued after sweep_tail(1,qb) so psy reuses the two
    just-freed ps_o PSUM slots; finalize(0) is HELD BACK to the drain
    and interleaved between the final flushes (fills the dead time while
    the last recip chain is in flight); its yb copies stay off DVE so
    the last scr/recip chain isn't queued behind them
"""

import numpy as np

B, N, D, H, HD = 2, 4096, 512, 8, 64
NT = N // 128          # 32 n-tiles
NBLK = N // 512        # 8 n-blocks
KC = N // 128          # 32 key chunks
QB = N // 512          # 8 q-blocks
SUPW = 2               # chunks per super


def _supers(unpair=False):
    if not unpair:
        return [[c, c + 1] for c in range(0, KC, 2)]
    # adjacent score MMs get SAME-parity chunks -> no half-array pairing
    out = []
    for c in range(0, KC, 4):
        out.append([c, c + 2])
        out.append([c + 1, c + 3])
    return out

# exp engine per unit, cycling: 'A' = ACT activation, 'D' = DVE fast-exp
EXP_PAT = "ADADADADADADADADA"   # 9A:8D per 17 units ~ 0.53 ACT share
# EXP_SPLIT: every unit's exp runs on BOTH engines (ACT cols [0,AQ), DVE
# rest) -> halves the scores->exp->attnv round-trip latency that gates the
# PSUM score-buffer ring, at the cost of one extra instruction per unit
EXP_SPLIT = False
AQ = 288           # ACT column share under EXP_SPLIT (DVE is ~1.25x slower)
YB_PAT = "AD"      # output-projection PSUM->SBUF drain engines, cycling
SCR_PAT = "D"      # attn-out PSUM->SBUF drain engines, cycling
LOOKAHEAD = 3
MUL_ON_POOL = True  # ot2 *= recip on gpsimd
# fast-exp constants: bits = rne(s * 0.125/ln2 * 1024 + 15*1024 + CORR)
FEXP_A = float(0.125 / np.log(2.0) * 1024.0)
FEXP_B = float(15.0 * 1024.0 - 15.0)

_CACHE = {}
ABLATE = "base"  # timing-ablation knob, used only by ablate.py


def _build_nc(loop_n=None):
    """Build the SPMD kernel. loop_n wraps the body in a hardware For loop
    (used only for timing amplification, never for the graded path)."""
    import contextlib

    import concourse.bass as bass
    import concourse.mybir as mybir
    import concourse.tile as tile
    from concourse import bacc

    F32 = mybir.dt.float32
    F32R = mybir.dt.float32r
    BF16 = mybir.dt.bfloat16
    F16 = mybir.dt.float16
    I16 = mybir.dt.int16
    EXP = mybir.ActivationFunctionType.Exp
    MULT = mybir.AluOpType.mult
    ADD = mybir.AluOpType.add

    nc = bacc.Bacc("TRN2", target_bir_lowering=False, debug=False, num_devices=8)

    xT_d = nc.dram_tensor("xT", [128, 4, N], BF16, kind="ExternalInput")
    w_d = {}
    for nm in ("wq2", "wk2", "wv"):
        w_d[nm] = nc.dram_tensor(nm, [128, 4, 128], BF16, kind="ExternalInput")
    woT_d = nc.dram_tensor("woT", [128, 512], F16, kind="ExternalInput")
    ident_d = nc.dram_tensor("ident", [128, 128], F16, kind="ExternalInput")
    y_d = nc.dram_tensor("y_part", [N, D], F16, kind="ExternalOutput")
    sums_dram = nc.dram_tensor("sums_scratch", [2, N], F16, kind="Internal")
    recip_dram = nc.dram_tensor("recip_scratch", [2, N], F16, kind="Internal")

    with tile.TileContext(nc) as tc:
        with (
            tc.tile_pool(name="singles", bufs=1) as singles,
            tc.tile_pool(name="sb_vt", bufs=2) as sb_vt,
            tc.tile_pool(name="sb_exp", bufs=6) as sb_exp,
            tc.tile_pool(name="sb_rsp", bufs=2) as sb_rsp,
            tc.tile_pool(name="sb_y", bufs=3) as sb_y,
            tc.tile_pool(name="ps3", bufs=3, space="PSUM") as ps3,
            tc.tile_pool(name="ps1", bufs=2, space="PSUM") as ps1,
        ):
            loop_ctx = (
                tc.For_i(0, loop_n, 1) if loop_n else contextlib.nullcontext()
            )
            with loop_ctx:
                # xT block 0 first: everything in phase 1 waits on it
                xT_all = singles.tile([128, 4, N], BF16, tag="xT", name="xT_all")
                nc.sync.dma_start(
                    out=xT_all[:, :, 0:512], in_=xT_d.ap()[:, :, 0:512]
                )
                ident = singles.tile([128, 128], F16)
                nc.sync.dma_start(out=ident, in_=ident_d.ap())
                # warm the ACT Exp table while phase 1 runs
                warm = singles.tile([1, 1], F32)
                nc.scalar.activation(out=warm, in_=ident[0:1, 0:1], func=EXP)
                wt = {}
                for nm in ("wq2", "wk2", "wv"):
                    wt[nm] = singles.tile(
                        [128, 4, 128], BF16, tag=f"w_{nm}", name=f"wt_{nm}"
                    )
                    nc.sync.dma_start(out=wt[nm], in_=w_d[nm].ap())
                woT = singles.tile([128, 512], F16)
                nc.sync.dma_start(out=woT, in_=woT_d.ap())

                qrep = [singles.tile([128, N], F16, tag=f"qrep{h}", name=f"qrep{h}")
                        for h in range(2)]
                krep = [singles.tile([128, N], F16, tag=f"krep{h}", name=f"krep{h}")
                        for h in range(2)]
                v_aug = [singles.tile([128, KC, 65], F16, tag=f"vaug{h}",
                                      name=f"vaug{h}") for h in range(2)]
                ot2 = singles.tile([128, N], F16)
                recip_b = singles.tile([128, N], F16)

                # ones column of v_aug (row 64 of attn@v = softmax sums)
                for h in range(2):
                    nc.vector.memset(v_aug[h][:, :, HD:65], 1.0)

                # -------- engine-cycling helpers --------
                def eng_copy(pat_state, pat):
                    e = pat[pat_state[0] % len(pat)]
                    pat_state[0] += 1
                    if e == "A":
                        return lambda out, in_: nc.scalar.copy(out=out, in_=in_)
                    return lambda out, in_: nc.vector.tensor_copy(out=out, in_=in_)

                yb_state = [0]
                scr_state = [0]

                # -------- attention helpers --------
                def norm_mul(qb, on_pool=MUL_ON_POOL):
                    qs = slice(qb * 512, (qb + 1) * 512)
                    eng = nc.gpsimd if on_pool else nc.vector
                    eng.tensor_mul(ot2[:, qs], ot2[:, qs], recip_b[:, qs])

                def finalize_nt(nt, from_ps3=False, yb_pat=None):
                    if from_ps3:
                        psy = ps3.tile([128, SUPW, 512], F32, tag="ps_s",
                                       name="psy3")[:, 0, :]
                    else:
                        psy = ps1.tile([128, 512], F32, tag="psA", name="psy")
                    nc.tensor.matmul(
                        psy, ot2[:, nt * 128:(nt + 1) * 128], woT,
                        start=True, stop=True,
                    )
                    yb = sb_y.tile([128, 512], F16, tag="yb", name="yb")
                    eng_copy(yb_state, yb_pat or YB_PAT)(out=yb, in_=psy)
                    nc.sync.dma_start(
                        out=y_d.ap()[nt * 128:(nt + 1) * 128, :], in_=yb
                    )

                def finalize(qb, mul=True, mul_on_pool=MUL_ON_POOL,
                             from_ps3=False):
                    if mul:
                        norm_mul(qb, mul_on_pool)
                    for nt in range(4 * qb, 4 * qb + 4):
                        finalize_nt(nt, from_ps3)

                def scores_exp(h, qb, chunks, use_dve):
                    qs = slice(qb * 512, (qb + 1) * 512)
                    ps_s = ps3.tile([128, SUPW, 512], F32, tag="ps_s", name="ps_s")
                    w = len(chunks)
                    for i, c in enumerate(chunks):
                        # fixed parity c%2: chunks are processed in order, so
                        # adjacent score MMs still land on opposite PE halves
                        # (the pairing trick), but each K chunk now lives in
                        # ONE half of krep -> no K replication needed.
                        p = c % 2
                        half = slice(p * 64, p * 64 + 64)
                        nc.tensor.matmul(
                            ps_s[:, i, :],
                            krep[h][half, c * 128:(c + 1) * 128],
                            qrep[h][half, qs],
                            start=True, stop=True,
                        )
                    expT = sb_exp.tile([128, SUPW, 512], F16, tag="expT",
                                       name="expT")
                    sl = slice(0, 1) if ABLATE == "tiny_exp" else slice(0, 512)
                    if use_dve and ABLATE != "all_act":
                        nc.vector.tensor_scalar(
                            out=expT.bitcast(I16)[:, 0:w, sl],
                            in0=ps_s[:, 0:w, sl],
                            scalar1=FEXP_A, scalar2=FEXP_B, op0=MULT, op1=ADD,
                        )
                    else:
                        nc.scalar.activation(
                            out=expT[:, 0:w, sl], in_=ps_s[:, 0:w, sl],
                            func=EXP, scale=0.125,
                        )
                    return expT

                def attnv(h, qb, ps_o, expT, chunks):
                    for i, c in enumerate(chunks):
                        if ABLATE == "no_attnv" and c > 0:
                            continue
                        nc.tensor.matmul(
                            ps_o[0:65, :], v_aug[h][:, c, :], expT[:, i, :],
                            start=(c == 0),
                            stop=(c == (0 if ABLATE == "no_attnv" else KC - 1)),
                        )

                def sweep_tail(h, qb, ps_o):
                    qs = slice(qb * 512, (qb + 1) * 512)
                    scr = sb_exp.tile([65, 512], F16, tag="scr", name="scr")
                    # scale by 1/16 so the unnormalized numerator fits f16
                    # (dominant-key rows reach ~1e5); the sums row is scaled
                    # identically, so its reciprocal cancels the factor
                    e = SCR_PAT[scr_state[0] % len(SCR_PAT)]
                    scr_state[0] += 1
                    if e == "A":
                        nc.scalar.activation(
                            out=scr, in_=ps_o[0:65, :],
                            func=mybir.ActivationFunctionType.Copy, scale=0.0625,
                        )
                    else:
                        nc.vector.tensor_scalar_mul(
                            out=scr, in0=ps_o[0:65, :], scalar1=0.0625
                        )
                    nc.sync.dma_start(
                        out=ot2[h * 64:(h + 1) * 64, qs], in_=scr[0:64, :]
                    )
                    # softmax sums: bounce via DRAM to spread the 512 values
                    # across 128 partitions (a [1,512] reciprocal would use a
                    # single DVE lane: measured 3.3us each), take the
                    # reciprocal at [128,4], bounce again to broadcast across
                    # partitions.
                    rrow = scr[64:65, :]
                    nc.sync.dma_start(out=sums_dram.ap()[h:h + 1, qs], in_=rrow)
                    rsp = sb_rsp.tile([128, 4], F16, tag="rsp", name="rsp")
                    rs_ap = bass.AP(
                        tensor=sums_dram, offset=h * N + qb * 512,
                        ap=[[4, 128], [1, 4]],
                    )
                    nc.sync.dma_start(out=rsp, in_=rs_ap)
                    with nc.allow_low_precision("f16 softmax-sum recip: 1e-3 "
                                                "rel err is within budget"):
                        nc.vector.reciprocal(out=rsp, in_=rsp)
                    rd_ap = bass.AP(
                        tensor=recip_dram, offset=h * N + qb * 512,
                        ap=[[4, 128], [1, 4]],
                    )
                    nc.sync.dma_start(out=rd_ap, in_=rsp)
                    rb = bass.AP(
                        tensor=recip_dram, offset=h * N + qb * 512,
                        ap=[[0, 64], [1, 512]],
                    )
                    nc.sync.dma_start(out=recip_b[h * 64:(h + 1) * 64, qs], in_=rb)
                    # psy reuses the two ps_o slots both heads just freed.
                    # finalize(0) is HELD BACK until the drain tail: it is PE
                    # work with long-satisfied deps, so it fills the dead time
                    # while the last q-block's softmax-sum recip chain (4
                    # serial DMAs) is in flight. Its normalize-multiply is
                    # issued early (here) so the tail sees pure PE work.
                    if h == 1:
                        if qb == 0:
                            norm_mul(0)
                        elif qb > 1:
                            finalize(qb - 1)

                # ---- phase 2 unit list (issued interleaved with phase 1) ----
                SUPERS = _supers(unpair=(ABLATE == "unpair"))
                units = []
                ui_pat = 0
                n_units = QB * len(SUPERS) * 2
                qb_range = [] if ABLATE == "phase1_only" else range(QB)
                for qb in qb_range:
                    for si, chunks in enumerate(SUPERS):
                        for h in range(2):
                            use_dve = EXP_PAT[ui_pat % len(EXP_PAT)] == "D"
                            # the last few units' exps go to ACT so DVE is
                            # free for the final sweep's scr/recip chain
                            if ui_pat >= n_units - 4:
                                use_dve = False
                            units.append(
                                (h, qb, chunks, si == len(SUPERS) - 1, use_dve)
                            )
                            ui_pat += 1

                ps_o_cur = {}
                pending = []

                def flush_pending():
                    h, qb, chunks, last, expT = pending.pop(0)
                    if (h, qb) not in ps_o_cur:
                        ps_o_cur[(h, qb)] = ps1.tile(
                            [128, 512], F32, tag="psA", name="ps_o"
                        )
                    attnv(h, qb, ps_o_cur[(h, qb)], expT, chunks)
                    if last:
                        sweep_tail(h, qb, ps_o_cur.pop((h, qb)))

                def issue_unit(u):
                    h, qb, chunks, last, use_dve = u
                    expT = scores_exp(h, qb, chunks, use_dve)
                    if len(pending) >= LOOKAHEAD:
                        flush_pending()
                    pending.append((h, qb, chunks, last, expT))

                def issue_pair(u1, u2):
                    # both heads' score MMs back-to-back (4 MMs alternating
                    # row halves, every Ldweights pulls ahead), then two
                    # attnv batches -> half as many score<->attnv row-group
                    # transitions on the PE
                    h1, qb1, chunks1, last1, dve1 = u1
                    h2, qb2, chunks2, last2, dve2 = u2
                    e1 = scores_exp(h1, qb1, chunks1, dve1)
                    e2 = scores_exp(h2, qb2, chunks2, dve2)
                    for _ in range(2):
                        if len(pending) >= LOOKAHEAD:
                            flush_pending()
                    pending.append((h1, qb1, chunks1, last1, e1))
                    pending.append((h2, qb2, chunks2, last2, e2))

                # unit readiness: block holding its q-block and k/v chunks
                def ready_block(u):
                    h, qb, chunks, last, use_dve = u
                    return max(qb, max(chunks) // 4)

                # ---- phase 1: load host-transposed x, project ----
                # Units whose inputs finished a block ago are interleaved
                # between the projection blocks: phase 2 is PE-hungry and
                # phase 1 is DMA/copy-latency-bound, so the early q-block-0
                # sweeps fill phase 1's stalls. (One block of slack so unit
                # score MMs never head-of-line-block the next projection.)
                ui = 0
                for jj in range(NBLK):
                    ns = slice(jj * 512, (jj + 1) * 512)
                    if jj > 0:
                        nc.sync.dma_start(
                            out=xT_all[:, :, ns], in_=xT_d.ap()[:, :, ns]
                        )
                    # Q and K for both heads. Q is replicated into both
                    # krep-halves by TWO ACT copies from the same PSUM rows
                    # (no SBUF->SBUF DMA); K needs no replication at all:
                    # chunk c is only ever read at partition-half c%2, so two
                    # strided DVE copies place even chunks in the top half and
                    # odd chunks in the bottom half.
                    for pi, (nm, rep) in enumerate((("wq2", qrep), ("wk2", krep))):
                        psp = ps3.tile([128, SUPW, 512], F32, tag="ps_s",
                                       name="psp")
                        for dc in range(4):
                            nc.tensor.matmul(
                                psp[:, 0, :], wt[nm][:, dc, :], xT_all[:, dc, ns],
                                start=(dc == 0), stop=(dc == 3),
                            )
                        if nm == "wq2":
                            for h in range(2):
                                src = psp[h * 64:(h + 1) * 64, 0, :]
                                nc.scalar.copy(out=rep[h][0:64, ns], in_=src)
                                nc.scalar.copy(out=rep[h][64:128, ns], in_=src)
                        else:
                            # block jj covers chunks 4jj..4jj+3: even chunks
                            # at cols {0:128, 256:384}, odd at {128:256,
                            # 384:512} within the block
                            for h in range(2):
                                srcs = psp[h * 64:(h + 1) * 64, 0, :].rearrange(
                                    "p (a b c) -> p a b c", a=2, b=2, c=128)
                                dst = rep[h][:, ns].rearrange(
                                    "p (a b c) -> p a b c", a=2, b=2, c=128)
                                nc.vector.tensor_copy(
                                    out=dst[0:64, :, 0, :], in_=srcs[:, :, 0, :]
                                )
                                nc.vector.tensor_copy(
                                    out=dst[64:128, :, 1, :], in_=srcs[:, :, 1, :]
                                )
                    # V for both heads, fp16, then transpose into v_aug
                    psp = ps3.tile([128, SUPW, 512], F32, tag="ps_s", name="psp_v")
                    for dc in range(4):
                        nc.tensor.matmul(
                            psp[:, 0, :], wt["wv"][:, dc, :], xT_all[:, dc, ns],
                            start=(dc == 0), stop=(dc == 3),
                        )
                    vt_blk = sb_vt.tile([128, 512], F16, tag="vt", name="vt_blk")
                    nc.vector.tensor_copy(out=vt_blk, in_=psp[:, 0, :])
                    for h in range(2):
                        psv = ps3.tile([128, 4, 64], F16, tag="ps_s", name="psv")
                        for tt in range(4):
                            nc.tensor.transpose(
                                psv[:, tt, :],
                                vt_blk[h * 64:(h + 1) * 64, tt * 128:(tt + 1) * 128],
                                ident[h * 64:(h + 1) * 64, h * 64:(h + 1) * 64],
                            )
                        nc.vector.tensor_copy(
                            out=v_aug[h][:, 4 * jj:4 * jj + 4, 0:HD], in_=psv
                        )

                    # phase 2 units whose inputs completed a block ago
                    # (units come in (h0,h1) pairs with identical readiness)
                    while (ui + 1 < len(units)
                           and ready_block(units[ui + 1]) <= jj - 1):
                        issue_pair(units[ui], units[ui + 1])
                        ui += 2

                # ---- phase 2: remaining sweeps, software-pipelined on PE ----
                # Heads interleaved (h innermost): independent dependency
                # chains keep the PE busy while the other head's exp is in
                # flight.
                while ui + 1 < len(units):
                    issue_pair(units[ui], units[ui + 1])
                    ui += 2
                while ui < len(units):
                    issue_unit(units[ui])
                    ui += 1
                # drain: interleave finalize(0)'s held-back output-projection
                # matmuls (deps long satisfied; psy tiles borrowed from the
                # now-draining ps3 scores ring) between the final flushes so
                # the PE stays busy while the last exps/sweeps are in flight.
                # fill yb copies stay OFF DVE so the last sweeps' scr/recip
                # chain isn't queued behind them
                fill = [
                    (lambda nt=nt: finalize_nt(nt, from_ps3=True, yb_pat="A"))
                    for nt in range(4)
                ] if ABLATE != "phase1_only" else []
                while pending:
                    flush_pending()
                    if fill:
                        fill.pop(0)()
                while fill:
                    fill.pop(0)()
                if ABLATE != "phase1_only":
                    finalize(QB - 1, mul_on_pool=False)

    nc.compile()
    return nc


def _prep_in_maps(x, Wq, Wk, Wv, Wo):
    import ml_dtypes

    bf16 = ml_dtypes.bfloat16
    x = np.asarray(x, dtype=np.float32)
    Wq = np.asarray(Wq, dtype=np.float32)
    Wk = np.asarray(Wk, dtype=np.float32)
    Wv = np.asarray(Wv, dtype=np.float32)
    Wo = np.asarray(Wo, dtype=np.float32)
    ident = np.eye(128, dtype=np.float16)
    in_maps = []
    for c in range(8):
        b, j = c // 4, c % 4
        rows = slice(128 * j, 128 * (j + 1))
        # xT layout [p, dc, n] with element [p, dc, n] = x[b][n, dc*128+p]
        xT = np.ascontiguousarray(
            x[b].T.reshape(4, 128, N).transpose(1, 0, 2)
        ).astype(bf16)
        m = {
            "xT": xT,
            "ident": ident,
            "woT": np.ascontiguousarray(Wo[:, rows].T).astype(np.float16),
        }
        for nm, W in (("wq2", Wq), ("wk2", Wk), ("wv", Wv)):
            A = W[rows]                     # [128, 512] rows = h1(64) | h2(64)
            # lhsT layout [p=d-within-chunk, c=d-chunk, k=out-col], contiguous
            m[nm] = np.ascontiguousarray(
                A.reshape(128, 4, 128).transpose(2, 1, 0)
            ).astype(bf16)
        in_maps.append(m)
    return in_maps


def _make_runner(nc):
    """Persistent jitted SPMD executor (mirrors bass2jax.run_bass_via_pjrt)
    so repeat kernel() calls skip re-tracing/re-jitting."""
    import jax
    from jax.sharding import Mesh, PartitionSpec
    from jax.experimental.shard_map import shard_map
    import concourse.mybir as mybir
    from concourse import bass2jax

    bass2jax.install_neuronx_cc_hook()
    n_cores = 8
    partition_name = nc.partition_id_tensor.name if nc.partition_id_tensor else None
    in_names, out_names, out_avals, zero_shapes = [], [], [], []
    for alloc in nc.m.functions[0].allocations:
        if not isinstance(alloc, mybir.MemoryLocationSet):
            continue
        name = alloc.memorylocations[0].name
        if alloc.kind == "ExternalInput":
            if name != partition_name:
                in_names.append(name)
        elif alloc.kind == "ExternalOutput":
            shape = tuple(alloc.tensor_shape)
            dtype = mybir.dt.np(alloc.dtype)
            out_names.append(name)
            out_avals.append(jax.core.ShapedArray(shape, dtype))
            zero_shapes.append((shape, dtype))
    n_params = len(in_names)
    all_names = in_names + out_names
    if partition_name is not None:
        all_names = all_names + [partition_name]

    def _body(*args):
        operands = list(args)
        if partition_name is not None:
            operands.append(bass2jax.partition_id_tensor())
        return tuple(bass2jax._bass_exec_p.bind(
            *operands,
            out_avals=tuple(out_avals),
            in_names=tuple(all_names),
            out_names=tuple(out_names),
            lowering_input_output_aliases=(),
            sim_require_finite=True,
            sim_require_nnan=True,
            nc=nc,
        ))

    mesh = Mesh(np.asarray(jax.devices()[:n_cores]), ("core",))
    n_outs = len(out_names)
    sharded = jax.jit(
        shard_map(
            _body, mesh=mesh,
            in_specs=(PartitionSpec("core"),) * (n_params + n_outs),
            out_specs=(PartitionSpec("core"),) * n_outs,
            check_rep=False,
        ),
        keep_unused=True,
    )
    zeros = [np.zeros((n_cores * s[0], *s[1:]), d) for s, d in zero_shapes]

    def run(in_maps):
        concat_in = [
            np.concatenate([np.asarray(in_maps[c][nm]) for c in range(n_cores)],
                           axis=0)
            for nm in in_names
        ]
        outs = sharded(*concat_in, *zeros)
        arr = np.asarray(outs[out_names.index("y_part")])
        return arr.reshape(n_cores, N, D)

    return run


def kernel(x, Wq, Wk, Wv, Wo, bo):
    if "nc" not in _CACHE:
        _CACHE["nc"] = _build_nc()
    nc = _CACHE["nc"]
    in_maps = _prep_in_maps(x, Wq, Wk, Wv, Wo)
    try:
        if "runner" not in _CACHE:
            _CACHE["runner"] = _make_runner(nc)
        parts = _CACHE["runner"](in_maps)
    except Exception:
        from concourse.bass_utils import run_bass_kernel_spmd
        res = run_bass_kernel_spmd(nc, in_maps, core_ids=list(range(8)))
        parts = np.stack([res.results[c]["y_part"] for c in range(8)])
    y = np.zeros((B, N, D), np.float32)
    for c in range(8):
        y[c // 4] += np.asarray(parts[c], dtype=np.float32)
    y += np.asarray(bo, dtype=np.float32)[None, None, :]
    return y


# revision 44
# speedup vs baseline: 1.1584x; 1.0830x over previous
"""Trainium2 Bass kernel for nn_MultiHeadAttention (b=2, n=4096, d=512, h=8, hd=64).

Sharding: 8 cores; core c handles batch b=c//4 and head pair j=c%4
(heads 2j, 2j+1). Tensor-parallel heads: each core computes a partial
output-projection y_part (f16); host sums the 4 partials per batch and
adds bo.

HW model (established by ablation timing, not the cost-model sim):
  - phase 2 is PE-BOUND: ~1100 512-col matmuls at ~213ns stream +
    ~40-70ns issue/LS overhead each. exp is nearly free (tiny_exp
    ablation: -27us) and all_act ~= split, so ACT/DVE loads don't bind.
  - PE MATMULs never execute concurrently ("pairing" is a myth — the PE
    reorder window only pulls LDWEIGHTS ahead, MATMULs stay in program
    order). But the LDWEIGHTS pull-ahead needs non-conflicting row
    groups: the unpair ablation (same-half consecutive score MMs) costs
    +111us. Hence the strict row-half ALTERNATION below.
  - scores use K=64 (half the array rows); alternating chunks between
    partition halves (chunk c lives at half c%2) keeps every score
    LDWEIGHTS pull-ahead eligible AND removes K replication entirely.

Per-core pipeline (16-bit datapaths):
  x[b] transposed ON HOST -> xT [128, 4dc, 4096] bf16, DMA'd straight to
    SBUF (no PE transpose / PSUM round trip in phase 1)
  Q/K projected bf16, drained to f16 qrep (replicated to both halves by
    two ACT copies) / f16 krep (parity-placed by strided DVE copies);
    f16 weights enable FWL and avoid the f32r LDWEIGHTS hazards
  V copied to fp16, PE-transposed into fp16 ones-augmented v_aug
    (M=65; row 64 of attn@v accumulates the softmax sums)
  scoresT supers of TWO chunks -> 2-bank PSUM tiles x3 bufs; fp16 expT
    via EITHER
    - ACT: one Exp ACTIVATE (scale 0.125 folded), fp16 out, or
    - DVE: Schraudolph fast-exp in ONE tensor_scalar f32->int16
      (bits = s*0.125*1024/ln2 + 15*1024 - 15), bitcast to fp16
    per-unit engine chosen by EXP_PAT
  attn@v: fp16 x fp16 matmuls accumulating [65, 512] f32 in PSUM
  per-q-block normalize: ps_o drained x1/16 to f16 (unnormalized
    numerator can reach ~1e5 > f16 max; sums row scales identically so
    the reciprocal cancels the 1/16), softmax sums bounce through DRAM
    (f16) to spread 512 values over 128 partitions, reciprocal at
    [128,4], bounce back to a partition-broadcast; y = OT.T @ WoT (f16)
    deferred one q-block; the ot2*recip multiply runs on gpsimd (Pool)
    which is otherwise idle (no PSUM port, so it can't help elsewhere)

Scheduling:
  - phase 2 units are interleaved INTO phase 1: each projection block
    enables 4 q-block-0 units (issued one block later so they never
    head-of-line-block the next projection); phase 1 alone is ~62us on
    HW and mostly DMA/copy latency, which phase-2 PE work now fills
  - units issued in (h0,h1) PAIRS: 4 score MMs back-to-back (alternating
    row halves), then two attnv batches -> fewer score<->attnv row-group
    transitions where LDWEIGHTS cannot pull ahead
  - finalize(qb-1) issued after sweep_tail(1,qb) so psy reuses the two
    just-freed ps_o PSUM slots; finalize(0) is HELD BACK to the drain
    and interleaved between the final flushes (fills the dead time while
    the last recip chain is in flight); its yb copies stay off DVE so
    the last scr/recip chain isn't queued behind them
"""

import numpy as np

B, N, D, H, HD = 2, 4096, 512, 8, 64
NT = N // 128          # 32 n-tiles
NBLK = N // 512        # 8 n-blocks
KC = N // 128          # 32 key chunks
QB = N // 512          # 8 q-blocks
SUPW = 2               # chunks per super


def _supers(unpair=False):
    if not unpair:
        return [[c, c + 1] for c in range(0, KC, 2)]
    # adjacent score MMs get SAME-parity chunks -> no half-array pairing
    out = []
    for c in range(0, KC, 4):
        out.append([c, c + 2])
        out.append([c + 1, c + 3])
    return out

# exp engine per unit, cycling: 'A' = ACT activation, 'D' = DVE fast-exp
EXP_PAT = "ADADADADADADADADA"   # 9A:8D per 17 units ~ 0.53 ACT share
# EXP_SPLIT: every unit's exp runs on BOTH engines (ACT cols [0,AQ), DVE
# rest) -> halves the scores->exp->attnv round-trip latency that gates the
# PSUM score-buffer ring, at the cost of one extra instruction per unit
EXP_SPLIT = False
AQ = 288           # ACT column share under EXP_SPLIT (DVE is ~1.25x slower)
YB_PAT = "AD"      # output-projection PSUM->SBUF drain engines, cycling
SCR_PAT = "D"      # attn-out PSUM->SBUF drain engines, cycling
LOOKAHEAD = 4
MUL_ON_POOL = True  # ot2 *= recip on gpsimd
# fast-exp constants: bits = rne(s * 0.125/ln2 * 1024 + 15*1024 + CORR)
FEXP_A = float(0.125 / np.log(2.0) * 1024.0)
FEXP_B = float(15.0 * 1024.0 - 15.0)

_CACHE = {}
ABLATE = "base"  # timing-ablation knob, used only by ablate.py


def _build_nc(loop_n=None):
    """Build the SPMD kernel. loop_n wraps the body in a hardware For loop
    (used only for timing amplification, never for the graded path)."""
    import contextlib

    import concourse.bass as bass
    import concourse.mybir as mybir
    import concourse.tile as tile
    from concourse import bacc

    F32 = mybir.dt.float32
    F32R = mybir.dt.float32r
    BF16 = mybir.dt.bfloat16
    F16 = mybir.dt.float16
    I16 = mybir.dt.int16
    EXP = mybir.ActivationFunctionType.Exp
    MULT = mybir.AluOpType.mult
    ADD = mybir.AluOpType.add

    nc = bacc.Bacc("TRN2", target_bir_lowering=False, debug=False, num_devices=8)

    xT_d = nc.dram_tensor("xT", [128, 4, N], BF16, kind="ExternalInput")
    w_d = {}
    for nm in ("wq2", "wk2", "wv"):
        w_d[nm] = nc.dram_tensor(nm, [128, 4, 128], BF16, kind="ExternalInput")
    woT_d = nc.dram_tensor("woT", [128, 512], F16, kind="ExternalInput")
    ident_d = nc.dram_tensor("ident", [128, 128], F16, kind="ExternalInput")
    y_d = nc.dram_tensor("y_part", [N, D], F16, kind="ExternalOutput")
    sums_dram = nc.dram_tensor("sums_scratch", [2, N], F16, kind="Internal")
    recip_dram = nc.dram_tensor("recip_scratch", [2, N], F16, kind="Internal")

    with tile.TileContext(nc) as tc:
        with (
            tc.tile_pool(name="singles", bufs=1) as singles,
            tc.tile_pool(name="sb_vt", bufs=2) as sb_vt,
            tc.tile_pool(name="sb_exp", bufs=8) as sb_exp,
            tc.tile_pool(name="sb_rsp", bufs=2) as sb_rsp,
            tc.tile_pool(name="sb_y", bufs=3) as sb_y,
            tc.tile_pool(name="ps3", bufs=3, space="PSUM") as ps3,
            tc.tile_pool(name="ps1", bufs=2, space="PSUM") as ps1,
        ):
            loop_ctx = (
                tc.For_i(0, loop_n, 1) if loop_n else contextlib.nullcontext()
            )
            with loop_ctx:
                # xT block 0 first: everything in phase 1 waits on it
                xT_all = singles.tile([128, 4, N], BF16, tag="xT", name="xT_all")
                nc.sync.dma_start(
                    out=xT_all[:, :, 0:512], in_=xT_d.ap()[:, :, 0:512]
                )
                ident = singles.tile([128, 128], F16)
                nc.sync.dma_start(out=ident, in_=ident_d.ap())
                # warm the ACT Exp table while phase 1 runs
                warm = singles.tile([1, 1], F32)
                nc.scalar.activation(out=warm, in_=ident[0:1, 0:1], func=EXP)
                wt = {}
                for nm in ("wq2", "wk2", "wv"):
                    wt[nm] = singles.tile(
                        [128, 4, 128], BF16, tag=f"w_{nm}", name=f"wt_{nm}"
                    )
                    nc.sync.dma_start(out=wt[nm], in_=w_d[nm].ap())
                woT = singles.tile([128, 512], F16)
                nc.sync.dma_start(out=woT, in_=woT_d.ap())

                qrep = [singles.tile([128, N], F16, tag=f"qrep{h}", name=f"qrep{h}")
                        for h in range(2)]
                krep = [singles.tile([128, N], F16, tag=f"krep{h}", name=f"krep{h}")
                        for h in range(2)]
                v_aug = [singles.tile([128, KC, 65], F16, tag=f"vaug{h}",
                                      name=f"vaug{h}") for h in range(2)]
                ot2 = singles.tile([128, N], F16)
                recip_b = singles.tile([128, N], F16)

                # ones column of v_aug (row 64 of attn@v = softmax sums)
                for h in range(2):
                    nc.vector.memset(v_aug[h][:, :, HD:65], 1.0)

                # -------- engine-cycling helpers --------
                def eng_copy(pat_state, pat):
                    e = pat[pat_state[0] % len(pat)]
                    pat_state[0] += 1
                    if e == "A":
                        return lambda out, in_: nc.scalar.copy(out=out, in_=in_)
                    return lambda out, in_: nc.vector.tensor_copy(out=out, in_=in_)

                yb_state = [0]
                scr_state = [0]

                # -------- attention helpers --------
                def norm_mul(qb, on_pool=MUL_ON_POOL):
                    qs = slice(qb * 512, (qb + 1) * 512)
                    eng = nc.gpsimd if on_pool else nc.vector
                    eng.tensor_mul(ot2[:, qs], ot2[:, qs], recip_b[:, qs])

                def finalize_nt(nt, from_ps3=False, yb_pat=None):
                    if from_ps3:
                        psy = ps3.tile([128, SUPW, 512], F32, tag="ps_s",
                                       name="psy3")[:, 0, :]
                    else:
                        psy = ps1.tile([128, 512], F32, tag="psA", name="psy")
                    nc.tensor.matmul(
                        psy, ot2[:, nt * 128:(nt + 1) * 128], woT,
                        start=True, stop=True,
                    )
                    yb = sb_y.tile([128, 512], F16, tag="yb", name="yb")
                    eng_copy(yb_state, yb_pat or YB_PAT)(out=yb, in_=psy)
                    nc.sync.dma_start(
                        out=y_d.ap()[nt * 128:(nt + 1) * 128, :], in_=yb
                    )

                def finalize(qb, mul=True, mul_on_pool=MUL_ON_POOL,
                             from_ps3=False):
                    if mul:
                        norm_mul(qb, mul_on_pool)
                    for nt in range(4 * qb, 4 * qb + 4):
                        finalize_nt(nt, from_ps3)

                def scores_exp(h, qb, chunks, use_dve):
                    qs = slice(qb * 512, (qb + 1) * 512)
                    ps_s = ps3.tile([128, SUPW, 512], F32, tag="ps_s", name="ps_s")
                    w = len(chunks)
                    for i, c in enumerate(chunks):
                        # fixed parity c%2: chunks are processed in order, so
                        # adjacent score MMs still land on opposite PE halves
                        # (the pairing trick), but each K chunk now lives in
                        # ONE half of krep -> no K replication needed.
                        p = c % 2
                        half = slice(p * 64, p * 64 + 64)
                        nc.tensor.matmul(
                            ps_s[:, i, :],
                            krep[h][half, c * 128:(c + 1) * 128],
                            qrep[h][half, qs],
                            start=True, stop=True,
                        )
                    expT = sb_exp.tile([128, SUPW, 512], F16, tag="expT",
                                       name="expT")
                    sl = slice(0, 1) if ABLATE == "tiny_exp" else slice(0, 512)
                    if use_dve and ABLATE != "all_act":
                        nc.vector.tensor_scalar(
                            out=expT.bitcast(I16)[:, 0:w, sl],
                            in0=ps_s[:, 0:w, sl],
                            scalar1=FEXP_A, scalar2=FEXP_B, op0=MULT, op1=ADD,
                        )
                    else:
                        nc.scalar.activation(
                            out=expT[:, 0:w, sl], in_=ps_s[:, 0:w, sl],
                            func=EXP, scale=0.125,
                        )
                    return expT

                def attnv(h, qb, ps_o, expT, chunks):
                    for i, c in enumerate(chunks):
                        if ABLATE == "no_attnv" and c > 0:
                            continue
                        nc.tensor.matmul(
                            ps_o[0:65, :], v_aug[h][:, c, :], expT[:, i, :],
                            start=(c == 0),
                            stop=(c == (0 if ABLATE == "no_attnv" else KC - 1)),
                        )

                def sweep_tail(h, qb, ps_o):
                    qs = slice(qb * 512, (qb + 1) * 512)
                    scr = sb_exp.tile([65, 512], F16, tag="scr", name="scr")
                    # scale by 1/16 so the unnormalized numerator fits f16
                    # (dominant-key rows reach ~1e5); the sums row is scaled
                    # identically, so its reciprocal cancels the factor
                    e = SCR_PAT[scr_state[0] % len(SCR_PAT)]
                    scr_state[0] += 1
                    if e == "A":
                        nc.scalar.activation(
                            out=scr, in_=ps_o[0:65, :],
                            func=mybir.ActivationFunctionType.Copy, scale=0.0625,
                        )
                    else:
                        nc.vector.tensor_scalar_mul(
                            out=scr, in0=ps_o[0:65, :], scalar1=0.0625
                        )
                    nc.sync.dma_start(
                        out=ot2[h * 64:(h + 1) * 64, qs], in_=scr[0:64, :]
                    )
                    # softmax sums: bounce via DRAM to spread the 512 values
                    # across 128 partitions (a [1,512] reciprocal would use a
                    # single DVE lane: measured 3.3us each), take the
                    # reciprocal at [128,4], bounce again to broadcast across
                    # partitions.
                    rrow = scr[64:65, :]
                    nc.sync.dma_start(out=sums_dram.ap()[h:h + 1, qs], in_=rrow)
                    rsp = sb_rsp.tile([128, 4], F16, tag="rsp", name="rsp")
                    rs_ap = bass.AP(
                        tensor=sums_dram, offset=h * N + qb * 512,
                        ap=[[4, 128], [1, 4]],
                    )
                    nc.sync.dma_start(out=rsp, in_=rs_ap)
                    with nc.allow_low_precision("f16 softmax-sum recip: 1e-3 "
                                                "rel err is within budget"):
                        nc.vector.reciprocal(out=rsp, in_=rsp)
                    rd_ap = bass.AP(
                        tensor=recip_dram, offset=h * N + qb * 512,
                        ap=[[4, 128], [1, 4]],
                    )
                    nc.sync.dma_start(out=rd_ap, in_=rsp)
                    rb = bass.AP(
                        tensor=recip_dram, offset=h * N + qb * 512,
                        ap=[[0, 64], [1, 512]],
                    )
                    nc.sync.dma_start(out=recip_b[h * 64:(h + 1) * 64, qs], in_=rb)
                    # psy reuses the two ps_o slots both heads just freed.
                    # finalize(0) is HELD BACK until the drain tail: it is PE
                    # work with long-satisfied deps, so it fills the dead time
                    # while the last q-block's softmax-sum recip chain (4
                    # serial DMAs) is in flight. Its normalize-multiply is
                    # issued early (here) so the tail sees pure PE work.
                    if h == 1:
                        if qb <= 1:
                            norm_mul(qb)
                        elif qb > 2:
                            finalize(qb - 1)

                # ---- phase 2 unit list (issued interleaved with phase 1) ----
                SUPERS = _supers(unpair=(ABLATE == "unpair"))
                units = []
                ui_pat = 0
                n_units = QB * len(SUPERS) * 2
                qb_range = [] if ABLATE == "phase1_only" else range(QB)
                for qb in qb_range:
                    for si, chunks in enumerate(SUPERS):
                        for h in range(2):
                            use_dve = EXP_PAT[ui_pat % len(EXP_PAT)] == "D"
                            # the last few units' exps go to ACT so DVE is
                            # free for the final sweep's scr/recip chain
                            if ui_pat >= n_units - 4:
                                use_dve = False
                            units.append(
                                (h, qb, chunks, si == len(SUPERS) - 1, use_dve)
                            )
                            ui_pat += 1

                ps_o_cur = {}
                pending = []

                def flush_pending():
                    h, qb, chunks, last, expT = pending.pop(0)
                    if (h, qb) not in ps_o_cur:
                        ps_o_cur[(h, qb)] = ps1.tile(
                            [128, 512], F32, tag="psA", name="ps_o"
                        )
                    attnv(h, qb, ps_o_cur[(h, qb)], expT, chunks)
                    if last:
                        sweep_tail(h, qb, ps_o_cur.pop((h, qb)))

                def issue_unit(u):
                    h, qb, chunks, last, use_dve = u
                    expT = scores_exp(h, qb, chunks, use_dve)
                    if len(pending) >= LOOKAHEAD:
                        flush_pending()
                    pending.append((h, qb, chunks, last, expT))

                def issue_pair(u1, u2):
                    # both heads' score MMs back-to-back (4 MMs alternating
                    # row halves, every Ldweights pulls ahead), then two
                    # attnv batches -> half as many score<->attnv row-group
                    # transitions on the PE
                    h1, qb1, chunks1, last1, dve1 = u1
                    h2, qb2, chunks2, last2, dve2 = u2
                    e1 = scores_exp(h1, qb1, chunks1, dve1)
                    e2 = scores_exp(h2, qb2, chunks2, dve2)
                    for _ in range(2):
                        if len(pending) >= LOOKAHEAD:
                            flush_pending()
                    pending.append((h1, qb1, chunks1, last1, e1))
                    pending.append((h2, qb2, chunks2, last2, e2))

                # unit readiness: block holding its q-block and k/v chunks
                def ready_block(u):
                    h, qb, chunks, last, use_dve = u
                    return max(qb, max(chunks) // 4)

                # ---- phase 1: load host-transposed x, project ----
                # Units whose inputs finished a block ago are interleaved
                # between the projection blocks: phase 2 is PE-hungry and
                # phase 1 is DMA/copy-latency-bound, so the early q-block-0
                # sweeps fill phase 1's stalls. (One block of slack so unit
                # score MMs never head-of-line-block the next projection.)
                ui = 0
                for jj in range(NBLK):
                    ns = slice(jj * 512, (jj + 1) * 512)
                    if jj > 0:
                        nc.sync.dma_start(
                            out=xT_all[:, :, ns], in_=xT_d.ap()[:, :, ns]
                        )
                    # Q and K for both heads. Q is replicated into both
                    # krep-halves by TWO ACT copies from the same PSUM rows
                    # (no SBUF->SBUF DMA); K needs no replication at all:
                    # chunk c is only ever read at partition-half c%2, so two
                    # strided DVE copies place even chunks in the top half and
                    # odd chunks in the bottom half.
                    for pi, (nm, rep) in enumerate((("wq2", qrep), ("wk2", krep))):
                        psp = ps3.tile([128, SUPW, 512], F32, tag="ps_s",
                                       name="psp")
                        for dc in range(4):
                            nc.tensor.matmul(
                                psp[:, 0, :], wt[nm][:, dc, :], xT_all[:, dc, ns],
                                start=(dc == 0), stop=(dc == 3),
                            )
                        if nm == "wq2":
                            for h in range(2):
                                src = psp[h * 64:(h + 1) * 64, 0, :]
                                nc.scalar.copy(out=rep[h][0:64, ns], in_=src)
                                nc.scalar.copy(out=rep[h][64:128, ns], in_=src)
                        else:
                            # block jj covers chunks 4jj..4jj+3: even chunks
                            # at cols {0:128, 256:384}, odd at {128:256,
                            # 384:512} within the block
                            for h in range(2):
                                srcs = psp[h * 64:(h + 1) * 64, 0, :].rearrange(
                                    "p (a b c) -> p a b c", a=2, b=2, c=128)
                                dst = rep[h][:, ns].rearrange(
                                    "p (a b c) -> p a b c", a=2, b=2, c=128)
                                nc.vector.tensor_copy(
                                    out=dst[0:64, :, 0, :], in_=srcs[:, :, 0, :]
                                )
                                nc.vector.tensor_copy(
                                    out=dst[64:128, :, 1, :], in_=srcs[:, :, 1, :]
                                )
                    # V for both heads, fp16, then transpose into v_aug
                    psp = ps3.tile([128, SUPW, 512], F32, tag="ps_s", name="psp_v")
                    for dc in range(4):
                        nc.tensor.matmul(
                            psp[:, 0, :], wt["wv"][:, dc, :], xT_all[:, dc, ns],
                            start=(dc == 0), stop=(dc == 3),
                        )
                    vt_blk = sb_vt.tile([128, 512], F16, tag="vt", name="vt_blk")
                    nc.vector.tensor_copy(out=vt_blk, in_=psp[:, 0, :])
                    for h in range(2):
                        psv = ps3.tile([128, 4, 64], F16, tag="ps_s", name="psv")
                        for tt in range(4):
                            nc.tensor.transpose(
                                psv[:, tt, :],
                                vt_blk[h * 64:(h + 1) * 64, tt * 128:(tt + 1) * 128],
                                ident[h * 64:(h + 1) * 64, h * 64:(h + 1) * 64],
                            )
                        nc.vector.tensor_copy(
                            out=v_aug[h][:, 4 * jj:4 * jj + 4, 0:HD], in_=psv
                        )

                    # phase 2 units whose inputs completed a block ago
                    # (units come in (h0,h1) pairs with identical readiness)
                    while (ui + 1 < len(units)
                           and ready_block(units[ui + 1]) <= jj - 1):
                        issue_pair(units[ui], units[ui + 1])
                        ui += 2

                # ---- phase 2: remaining sweeps, software-pipelined on PE ----
                # Heads interleaved (h innermost): independent dependency
                # chains keep the PE busy while the other head's exp is in
                # flight.
                # 3-unit bursts: 6 score MMs back-to-back, then 3 attnv
                # batches (ps3 ring has 3 slots, so 3 is the burst cap)
                while ui + 2 < len(units):
                    u1, u2, u3 = units[ui], units[ui + 1], units[ui + 2]
                    e1 = scores_exp(u1[0], u1[1], u1[2], u1[4])
                    e2 = scores_exp(u2[0], u2[1], u2[2], u2[4])
                    e3 = scores_exp(u3[0], u3[1], u3[2], u3[4])
                    for _ in range(3):
                        if len(pending) >= LOOKAHEAD:
                            flush_pending()
                    pending.append((u1[0], u1[1], u1[2], u1[3], e1))
                    pending.append((u2[0], u2[1], u2[2], u2[3], e2))
                    pending.append((u3[0], u3[1], u3[2], u3[3], e3))
                    ui += 3
                while ui < len(units):
                    issue_unit(units[ui])
                    ui += 1
                # drain: interleave finalize(0)'s held-back output-projection
                # matmuls (deps long satisfied; psy tiles borrowed from the
                # now-draining ps3 scores ring) between the final flushes so
                # the PE stays busy while the last exps/sweeps are in flight.
                # fill yb copies stay OFF DVE so the last sweeps' scr/recip
                # chain isn't queued behind them
                fill = [
                    (lambda nt=nt: finalize_nt(nt, from_ps3=True, yb_pat="A"))
                    for nt in range(8)
                ] if ABLATE != "phase1_only" else []
                while pending:
                    flush_pending()
                    if fill:
                        fill.pop(0)()
                while fill:
                    fill.pop(0)()
                if ABLATE != "phase1_only":
                    finalize(QB - 1, mul_on_pool=False)

    nc.compile()
    return nc


def _prep_in_maps(x, Wq, Wk, Wv, Wo):
    import ml_dtypes

    bf16 = ml_dtypes.bfloat16
    x = np.asarray(x, dtype=np.float32)
    Wq = np.asarray(Wq, dtype=np.float32)
    Wk = np.asarray(Wk, dtype=np.float32)
    Wv = np.asarray(Wv, dtype=np.float32)
    Wo = np.asarray(Wo, dtype=np.float32)
    ident = np.eye(128, dtype=np.float16)
    in_maps = []
    for c in range(8):
        b, j = c // 4, c % 4
        rows = slice(128 * j, 128 * (j + 1))
        # xT layout [p, dc, n] with element [p, dc, n] = x[b][n, dc*128+p]
        xT = np.ascontiguousarray(
            x[b].T.reshape(4, 128, N).transpose(1, 0, 2)
        ).astype(bf16)
        m = {
            "xT": xT,
            "ident": ident,
            "woT": np.ascontiguousarray(Wo[:, rows].T).astype(np.float16),
        }
        for nm, W in (("wq2", Wq), ("wk2", Wk), ("wv", Wv)):
            A = W[rows]                     # [128, 512] rows = h1(64) | h2(64)
            # lhsT layout [p=d-within-chunk, c=d-chunk, k=out-col], contiguous
            m[nm] = np.ascontiguousarray(
                A.reshape(128, 4, 128).transpose(2, 1, 0)
            ).astype(bf16)
        in_maps.append(m)
    return in_maps


def _make_runner(nc):
    """Persistent jitted SPMD executor (mirrors bass2jax.run_bass_via_pjrt)
    so repeat kernel() calls skip re-tracing/re-jitting."""
    import jax
    from jax.sharding import Mesh, PartitionSpec
    from jax.experimental.shard_map import shard_map
    import concourse.mybir as mybir
    from concourse import bass2jax

    bass2jax.install_neuronx_cc_hook()
    n_cores = 8
    partition_name = nc.partition_id_tensor.name if nc.partition_id_tensor else None
    in_names, out_names, out_avals, zero_shapes = [], [], [], []
    for alloc in nc.m.functions[0].allocations:
        if not isinstance(alloc, mybir.MemoryLocationSet):
            continue
        name = alloc.memorylocations[0].name
        if alloc.kind == "ExternalInput":
            if name != partition_name:
                in_names.append(name)
        elif alloc.kind == "ExternalOutput":
            shape = tuple(alloc.tensor_shape)
            dtype = mybir.dt.np(alloc.dtype)
            out_names.append(name)
            out_avals.append(jax.core.ShapedArray(shape, dtype))
            zero_shapes.append((shape, dtype))
    n_params = len(in_names)
    all_names = in_names + out_names
    if partition_name is not None:
        all_names = all_names + [partition_name]

    def _body(*args):
        operands = list(args)
        if partition_name is not None:
            operands.append(bass2jax.partition_id_tensor())
        return tuple(bass2jax._bass_exec_p.bind(
            *operands,
            out_avals=tuple(out_avals),
            in_names=tuple(all_names),
            out_names=tuple(out_names),
            lowering_input_output_aliases=(),
            sim_require_finite=True,
            sim_require_nnan=True,
            nc=nc,
        ))

    mesh = Mesh(np.asarray(jax.devices()[:n_cores]), ("core",))
    n_outs = len(out_names)
    sharded = jax.jit(
        shard_map(
            _body, mesh=mesh,
            in_specs=(PartitionSpec("core"),) * (n_params + n_outs),
            out_specs=(PartitionSpec("core"),) * n_outs,
            check_rep=False,
        ),
        keep_unused=True,
    )
    zeros = [np.zeros((n_cores * s[0], *s[1:]), d) for s, d in zero_shapes]

    def run(in_maps):
        concat_in = [
            np.concatenate([np.asarray(in_maps[c][nm]) for c in range(n_cores)],
                           axis=0)
            for nm in in_names
        ]
        outs = sharded(*concat_in, *zeros)
        arr = np.asarray(outs[out_names.index("y_part")])
        return arr.reshape(n_cores, N, D)

    return run


def kernel(x, Wq, Wk, Wv, Wo, bo):
    if "nc" not in _CACHE:
        _CACHE["nc"] = _build_nc()
    nc = _CACHE["nc"]
    in_maps = _prep_in_maps(x, Wq, Wk, Wv, Wo)
    try:
        if "runner" not in _CACHE:
            _CACHE["runner"] = _make_runner(nc)
        parts = _CACHE["runner"](in_maps)
    except Exception:
        from concourse.bass_utils import run_bass_kernel_spmd
        res = run_bass_kernel_spmd(nc, in_maps, core_ids=list(range(8)))
        parts = np.stack([res.results[c]["y_part"] for c in range(8)])
    y = np.zeros((B, N, D), np.float32)
    for c in range(8):
        y[c // 4] += np.asarray(parts[c], dtype=np.float32)
    y += np.asarray(bo, dtype=np.float32)[None, None, :]
    return y
